# revision 6
# baseline (speedup 1.0000x reference)
"""Trainium2 Bass kernel for nn_NeuralTensorLayer (bs=131072, d=k=64, 8 cores).

Math (all row-major reshapes are free reinterpretations of flat buffers):
  ntn(x, y, T):  btp[k,b] = sum_ij x[b,i] T[k,i,j] y[b,j]
                 ffp[k,c] = sum_r W[k,r] * concat(x,y).reshape(128, bs)[r, c]
                 out = tanh(btp + ffp + bias)            # (64, bs) flat
  a  = ntn(e1, p, T1);  c = ntn(p, e2, T2);  ar = ntn(er, p, T1)   (as (bs,64))
  u  = ntn(a, c, T3);  ur = ntn(ar, c, T3)   -> stack -> (2, 64, bs)

Kernel strategy (per core, batch-sharded 16384 rows, zero device-to-device comm):
  btp[b,k] = sum_i x[b,i] * Ptil[b, k*64+i],  Ptil = shared_operand @ G(T)
  - PE (float32r): Ptil tiles (128b x 2048) into PSUM, 64-contraction only.
  - DVE custom op ANT_MUL_SCAN: state += Src0*Src1 prefix-scan over the 2048
    stream (fp32). Segment sums recovered by strided extract + diff.
  - ffp: small PE matmul per tile with host-pregathered strided rows (Ahat).
  - tanh on ACT. Stage-1 outputs (bs,64)-natural; host transposes the shared
    operand (p / c) and re-gathers Ahat between the two launches.
"""

import threading

import numpy as np

import concourse.bass as bass
import concourse.bacc as bacc
import concourse.mybir as mybir
from concourse.tile import TileContext
from concourse.bass_utils import run_bass_kernel_spmd

# ---------------------------------------------------------------- constants
BS = 131072
D = 64
NCORES = 8
BSL = BS // NCORES          # 16384 rows per core
TILES = BSL // 128          # 128 b-tiles per call
HALF = 2048                 # scan stream width (4 PSUM banks)
SEGS = HALF // D            # 32 k-segments per scan
F32 = mybir.dt.float32
F32R = mybir.dt.float32r

# ------------------------------------------------------- custom DVE op
# out[p, t] = sum_{t' <= t} in0[p, t'] * in1[p, t']   (fp32 running state)
from concourse.dve_ops import (  # noqa: E402
    OPS,
    CUSTOM_DVE_SPECS,
    DveOp,
    _CUSTOM_DVE_ROW_BASE,
    _SUB_OPCODE_FOR_NAME,
)
from concourse.dve_spec import AluOp, Spec, Src0, Src1, lower  # noqa: E402
from concourse.dve_spec import scan as dve_scan  # noqa: E402
from concourse.dve_uop import DveOpSpec  # noqa: E402


def _ref_mul_scan(in0, in1, c0, c1, c2):
    return np.cumsum(in0.astype(np.float32) * in1.astype(np.float32), axis=-1).astype(
        np.float32
    )


def _make_mul_scan() -> DveOp:
    name = "ANT_MUL_SCAN"
    if name in _SUB_OPCODE_FOR_NAME:
        for op in OPS:
            if op.name == name:
                return op
    spec = Spec(body=dve_scan(AluOp.ADD, Src0 * Src1), reference=_ref_mul_scan)
    row = _CUSTOM_DVE_ROW_BASE + len(OPS)
    assert row < 0x20
    shas = {}
    for ver in ("v3", "v4"):
        s = DveOpSpec(name=name, opcode=row, uops=lower(spec, ver=ver), rd1_en=True)
        shas[ver] = s.sha(ver)
    op = DveOp(name, spec, subdim=False, uops_sha=shas)
    OPS.append(op)
    _SUB_OPCODE_FOR_NAME[name] = row
    CUSTOM_DVE_SPECS[name] = spec
    return op


MUL_SCAN = _make_mul_scan()

# ------------------------------------------------------- kernel program


def _build_stage(n_calls: int, g_map: list[int]):
    """One launch: for each call i, out_i = tanh(btp(lhsT, s1_i, G[g_map[i]])
    + ffp(Ahat_i, W) + bias). All tensors fp32."""
    n_g = max(g_map) + 1
    nc = bacc.Bacc("TRN2", target_bir_lowering=False, debug=False)

    lhsT_d = nc.dram_tensor("lhst", (D, BSL), F32R, kind="ExternalInput").ap()
    s1_d = [
        nc.dram_tensor(f"s1_{i}", (BSL, D), F32, kind="ExternalInput").ap()
        for i in range(n_calls)
    ]
    g_d = [
        nc.dram_tensor(f"g_{j}", (D, D * D), F32R, kind="ExternalInput").ap()
        for j in range(n_g)
    ]
    a_d = [
        nc.dram_tensor(f"ahat_{i}", (128, BSL), F32R, kind="ExternalInput").ap()
        for i in range(n_calls)
    ]
    wt_d = nc.dram_tensor("wt", (2 * D, D), F32R, kind="ExternalInput").ap()
    ones_d = nc.dram_tensor("ones", (1, 2 * D), F32R, kind="ExternalInput").ap()
    bt_d = nc.dram_tensor("bt", (1, D), F32R, kind="ExternalInput").ap()
    o_d = [
        nc.dram_tensor(f"o_{i}", (BSL, D), F32, kind="ExternalOutput").ap()
        for i in range(n_calls)
    ]

    with TileContext(nc) as tc:
        with (
            tc.tile_pool(name="const", bufs=1) as cpool,
            tc.tile_pool(name="lhst", bufs=1) as lpool,
            tc.tile_pool(name="ffp", bufs=1) as fpool,
            tc.tile_pool(name="work", bufs=3) as wpool,
            tc.tile_pool(name="scan", bufs=3) as spool,
            tc.tile_pool(name="stat", bufs=1) as stpool,
            tc.tile_pool(name="atile", bufs=3) as apool,
            tc.tile_pool(name="psum", bufs=2, space="PSUM") as pspool,
        ):
            # constants
            g_sb = []
            for j in range(n_g):
                gt = cpool.tile([D, D * D], F32R, tag=f"g{j}")
                nc.sync.dma_start(gt[:], g_d[j])
                g_sb.append(gt)
            wt_sb = cpool.tile([2 * D, D], F32R, tag="wt")
            nc.sync.dma_start(wt_sb[:], wt_d)
            bt_sb = cpool.tile([1, D], F32R, tag="bt")
            nc.sync.dma_start(bt_sb[:], bt_d)
            ones_sb = cpool.tile([1, 2 * D], F32R, tag="ones")
            nc.sync.dma_start(ones_sb[:], ones_d)
            lhsT_sb = lpool.tile([D, BSL], F32R, tag="lhsT")
            nc.sync.dma_start(lhsT_sb[:], lhsT_d)
            # static diff buffers (col 0 stays zero)
            buf0 = stpool.tile([128, SEGS + 1], F32, tag="buf0")
            buf1 = stpool.tile([128, SEGS + 1], F32, tag="buf1")
            bufs = [buf0, buf1]
            for h in range(2):
                nc.vector.memset(bufs[h][:], 0.0)

            for i in range(n_calls):
                gi = g_sb[g_map[i]]
                ffp_sb = fpool.tile([128, TILES * D], F32, tag="ffp")
                # --- ffp pre-pass: ffp_sb[:, t*64:] = Ahat_t.T @ W.T + b ---
                for t in range(TILES):
                    at = apool.tile([128, 128], F32R, tag="at")
                    nc.sync.dma_start(at[:], a_d[i][:, t * 128 : (t + 1) * 128])
                    psf = pspool.tile([128, HALF], F32, tag="ps")
                    nc.tensor.matmul(
                        psf[:, :D],
                        at[:],
                        wt_sb[:],
                        start=True,
                        stop=False,
                    )
                    nc.tensor.matmul(
                        psf[:, :D],
                        ones_sb[:],
                        bt_sb[:],
                        start=False,
                        stop=True,
                    )
                    nc.scalar.copy(ffp_sb[:, t * D : (t + 1) * D], psf[:, :D])
                # --- main pass ---
                for t in range(TILES):
                    s1t = wpool.tile([128, D], F32, tag="s1t")
                    nc.sync.dma_start(s1t[:], s1_d[i][t * 128 : (t + 1) * 128, :])
                    y = wpool.tile([128, D], F32, tag="y")
                    lt = lhsT_sb[:, t * 128 : (t + 1) * 128]
                    for h in range(2):
                        ps = pspool.tile([128, HALF], F32, tag="ps")
                        for n_ in range(HALF // 512):
                            nc.tensor.matmul(
                                ps[:, n_ * 512 : (n_ + 1) * 512],
                                lt,
                                gi[:, h * HALF + n_ * 512 : h * HALF + (n_ + 1) * 512],
                                start=True,
                                stop=True,
                            )
                        sc = spool.tile([128, HALF], F32, tag="sc")
                        nc.vector._custom_dve(
                            MUL_SCAN,
                            out=sc[:],
                            in0=ps[:],
                            in1=s1t[:, None, :].broadcast_to((128, SEGS, D)),
                        )
                        seg = sc[:].rearrange("p (s i) -> p s i", i=D)[:, :, D - 1]
                        nc.vector.tensor_copy(out=bufs[h][:, 1 : SEGS + 1], in_=seg)
                        nc.vector.tensor_sub(
                            y[:, h * SEGS : (h + 1) * SEGS],
                            bufs[h][:, 1 : SEGS + 1],
                            bufs[h][:, 0:SEGS],
                        )
                    nc.vector.tensor_add(y[:], y[:], ffp_sb[:, t * D : (t + 1) * D])
                    ot = wpool.tile([128, D], F32, tag="ot")
                    nc.scalar.activation(ot[:], y[:], mybir.ActivationFunctionType.Tanh)
                    nc.sync.dma_start(o_d[i][t * 128 : (t + 1) * 128, :], ot[:])
    nc.compile()
    return nc


_BUILD_LOCK = threading.Lock()
_CACHE: dict = {}


def _get_stage(n_calls: int, g_map: tuple):
    key = (n_calls, g_map)
    with _BUILD_LOCK:
        if key not in _CACHE:
            _CACHE[key] = _build_stage(n_calls, list(g_map))
    return _CACHE[key]


# ------------------------------------------------------- host-side helpers


def _ahat(x: np.ndarray, y: np.ndarray) -> list[np.ndarray]:
    """Per-core column-slices of ff_in = concat(x,y).reshape(2d, bs)."""
    ff = np.concatenate([x, y], axis=1).reshape(2 * D, BS)
    return [
        np.ascontiguousarray(ff[:, m * BSL : (m + 1) * BSL]) for m in range(NCORES)
    ]


def _unscramble(o: np.ndarray) -> np.ndarray:
    """Stage-1 device output o (bs, 64) holds o[b, k] = ntn_out[k, b].
    The reference's next-stage input is ntn_out.reshape(bs, d) — a flat
    reinterpretation of the (64, bs) matrix."""
    return np.ascontiguousarray(o.T).reshape(BS, D)


def _g_of(T: np.ndarray, mode: str) -> np.ndarray:
    if mode == "ki":   # G[j, k*64+i] = T[k,i,j]
        return np.ascontiguousarray(T.transpose(2, 0, 1).reshape(D, D * D))
    else:              # "kj": G[i, k*64+j] = T[k,i,j]
        return np.ascontiguousarray(T.transpose(1, 0, 2).reshape(D, D * D))


def _run_stage(nc, per_core_inputs: list[dict], trace=False):
    res = run_bass_kernel_spmd(
        nc, per_core_inputs, core_ids=list(range(NCORES)), trace=trace
    )
    return res


def _launch(nc, in_maps, n_outs, trace=False):
    r = _run_stage(nc, in_maps, trace=trace)
    outs = [
        np.concatenate([r.results[m][f"o_{i}"] for m in range(NCORES)], axis=0)
        for i in range(n_outs)
    ]
    return outs, r


# ------------------------------------------------------- public entry point

LAST_EXEC_NS = [None, None]
ONES = np.ones((1, 2 * D), np.float32)


def kernel(e1, p, e2, er, T1, T2, T3, W, b, trace=False):
    e1, p, e2, er = (np.ascontiguousarray(t, np.float32) for t in (e1, p, e2, er))
    T1, T2, T3 = (np.asarray(t, np.float32) for t in (T1, T2, T3))
    W = np.asarray(W, np.float32)
    b = np.asarray(b, np.float32)

    g1 = _g_of(T1, "ki")
    g2 = _g_of(T2, "kj")   # c-call contracts p over i, scans e2 over j
    g3 = _g_of(T3, "ki")
    wt = np.ascontiguousarray(W.T)
    bt = np.ascontiguousarray(b.reshape(1, D))

    nc1 = _get_stage(3, (0, 1, 0))
    nc2 = _get_stage(2, (0, 0))

    # ---- stage 1: a = (e1,p,G1), c = (p,e2,G2'), ar = (er,p,G1) ----
    pT = [np.ascontiguousarray(p[m * BSL : (m + 1) * BSL].T) for m in range(NCORES)]
    a1 = _ahat(e1, p)
    a2 = _ahat(p, e2)
    a3 = _ahat(er, p)
    in1 = []
    for m in range(NCORES):
        sl = slice(m * BSL, (m + 1) * BSL)
        in1.append(
            {
                "lhst": pT[m],
                "s1_0": np.ascontiguousarray(e1[sl]),
                "s1_1": np.ascontiguousarray(e2[sl]),
                "s1_2": np.ascontiguousarray(er[sl]),
                "g_0": g1,
                "g_1": g2,
                "ahat_0": a1[m],
                "ahat_1": a2[m],
                "ahat_2": a3[m],
                "wt": wt,
                "bt": bt,
                "ones": ONES,
            }
        )
    (o_a, o_c, o_ar), r1 = _launch(nc1, in1, 3, trace=trace)
    LAST_EXEC_NS[0] = r1.exec_time_ns
    a, c, ar = _unscramble(o_a), _unscramble(o_c), _unscramble(o_ar)

    # ---- stage 2: u = (a,c,G3), ur = (ar,c,G3) ----
    cT = [np.ascontiguousarray(c[m * BSL : (m + 1) * BSL].T) for m in range(NCORES)]
    au = _ahat(a, c)
    aur = _ahat(ar, c)
    in2 = []
    for m in range(NCORES):
        sl = slice(m * BSL, (m + 1) * BSL)
        in2.append(
            {
                "lhst": cT[m],
                "s1_0": np.ascontiguousarray(a[sl]),
                "s1_1": np.ascontiguousarray(ar[sl]),
                "g_0": g3,
                "ahat_0": au[m],
                "ahat_1": aur[m],
                "wt": wt,
                "bt": bt,
                "ones": ONES,
            }
        )
    (u, ur), r2 = _launch(nc2, in2, 2, trace=trace)
    LAST_EXEC_NS[1] = r2.exec_time_ns

    out = np.empty((2, D, BS), np.float32)
    out[0] = u.T
    out[1] = ur.T
    return out
